# revision 50
# baseline (speedup 1.0000x reference)
"""MoE-LoRA linear kernel for TRN2, data-parallel over tokens across 8 cores.

Per-core computation (Tc tokens, D=1024, E=10, R=4, TOP_K=2):
  base = x @ W^T + b ; logits = x @ gateW^T + gb ; top2 softmax -> dense w[t,e]
  h = (x @ lora_down^T) * w  (rank-expanded) ; out = base + 0.25 * h @ lora_up^T

Datapath is fp16 (x, W, gate, lora) with fp32 PSUM accumulate — end-to-end
rel err ~1.3e-3 vs the fp32 reference (validated on host). fp16 enables the
DMA xbar transpose, so x streams DRAM->SBUF already transposed per k-block:
no PE transposes, no psum->sbuf copies for xT, and half the x/W HBM traffic.
The down-projection and gate logits share one 50-column stationary matrix
(one matmul per k-block). Softmax stays fp32.

Emission is software-pipelined so the in-order Tensor queue never waits on
the Vector softmax: phase A of group g+2 and the gating transposes of group
g+1 are emitted inside phase C of group g. Inputs ride the Sync DMA queue;
weights + outputs ride the Scalar (Activation) queue.
"""

import contextlib
import ctypes
import sys
import types

import numpy as np

SO_PATH = "/opt/axon/libaxon_pjrt.so"

D = 1024
E = 10
R = 4
ER = E * R          # 40
DN0 = 32            # down block start (32-aligned partition base)
GCOLS = DN0 + ER    # 72 = gate(10) + pad + down(40)@32
GW = GCOLS + 1      # 73: col 72 of kb 0 carries gate_b (avoids a 10x4B
                    # scattered DMA that takes ~10us to complete)
TT_PER_GROUP = 4    # 128-token tiles per 512-token group
TG = 128 * TT_PER_GROUP  # 512 tokens per group


def install_ntff_hook():
    """run_bass_kernel_spmd(trace=True) needs antenv.axon_hooks; synthesize it."""
    if "antenv.axon_hooks" in sys.modules:
        return
    def _ntff_profile_via_ctypes(so_path):
        lib = ctypes.CDLL(so_path)
        if not hasattr(lib, "axon_start_nrt_profile"):
            return None
        lib.axon_start_nrt_profile.argtypes = [ctypes.POINTER(ctypes.c_int64), ctypes.c_size_t]
        lib.axon_start_nrt_profile.restype = ctypes.c_int64
        lib.axon_stop_nrt_profile.argtypes = [ctypes.c_char_p]
        lib.axon_stop_nrt_profile.restype = ctypes.c_int64

        @contextlib.contextmanager
        def _hook(output_dir, device_ids):
            import jax
            jax.devices()
            if device_ids:
                ids = (ctypes.c_int64 * len(device_ids))(*device_ids)
                rc = lib.axon_start_nrt_profile(ids, len(device_ids))
            else:
                rc = lib.axon_start_nrt_profile(None, 0)
            if rc != 0:
                raise RuntimeError(f"axon_start_nrt_profile rc={rc}")
            try:
                yield
            finally:
                n = lib.axon_stop_nrt_profile(str(output_dir).encode())
                if n < 0:
                    raise RuntimeError(f"axon_stop_nrt_profile rc={n}")
        return _hook

    mod = types.ModuleType("antenv.axon_hooks")
    mod.get_axon_ntff_profile_hook = lambda: _ntff_profile_via_ctypes(SO_PATH)
    sys.modules["antenv.axon_hooks"] = mod


def build_kernel(Tc, n_cores=8):
    import concourse.bass as bass  # noqa: F401
    import concourse.mybir as mybir
    import concourse.tile as tile
    from concourse import bacc
    from concourse.bass import ds, ts
    from concourse.masks import make_identity

    f16 = mybir.dt.float16
    f32 = mybir.dt.float32
    NG = Tc // TG  # groups of 512 tokens
    assert Tc % TG == 0

    nc = bacc.Bacc("TRN2", target_bir_lowering=False, debug=False, num_devices=n_cores)

    x_in = nc.declare_dram_parameter("x", [Tc, D], f16, isOutput=False)
    wt_in = nc.declare_dram_parameter("wt", [128, 8, D], f16, isOutput=False)
    g_in = nc.declare_dram_parameter("g", [128, 8, GW], f16, isOutput=False)
    u_in = nc.declare_dram_parameter("u", [ER + 1, D], f16, isOutput=False)
    out_dram = nc.declare_dram_parameter("out", [Tc, D], f32, isOutput=True)

    with tile.TileContext(nc) as tc:
        with contextlib.ExitStack() as ctx:
            singles = ctx.enter_context(tc.tile_pool(name="singles", bufs=1))
            smallp = ctx.enter_context(tc.tile_pool(name="smallp", bufs=2))
            h1p = ctx.enter_context(tc.tile_pool(name="h1p", bufs=3))
            outp = ctx.enter_context(tc.tile_pool(name="outp", bufs=4))
            pgp = ctx.enter_context(tc.tile_pool(name="pgp", bufs=1, space="PSUM"))
            pslp = ctx.enter_context(tc.tile_pool(name="pslp", bufs=1, space="PSUM"))
            pswp = ctx.enter_context(tc.tile_pool(name="pswp", bufs=1, space="PSUM"))
            pop = ctx.enter_context(tc.tile_pool(name="pop", bufs=3, space="PSUM"))
            ptp = ctx.enter_context(tc.tile_pool(name="ptp", bufs=2, space="PSUM"))

            # ---- constants / persistent tensors ----
            wt_sb = singles.tile([128, 8, D], f16)
            g_sb = singles.tile([128, 8, GW], f16)
            u_sb = singles.tile([ER + 1, D], f16)
            ident = singles.tile([128, 128], f32)
            ident16 = singles.tile([128, 128], f16)
            ones = singles.tile([ER + 1, 1], f32)
            xT = singles.tile([128, NG, 8, TG], f16)
            x_raw = singles.tile([128, NG, TT_PER_GROUP, D], f16)

            # inputs on the Sync DMA queue; weights+outputs on Scalar's queue
            # Plain contiguous loads only (DMA xbar transposes serialize on
            # completion and collapse under bandwidth contention): x tokens on
            # partitions via Sync queue; weights + outputs on Scalar queue.
            nc.sync.dma_start(out=g_sb[:], in_=g_in[:])
            xv = x_in.rearrange("(ng a p) d -> ng p a d", p=128, a=TT_PER_GROUP)
            # group 0 lands per-tt so the first transposes start ~10us sooner
            for tt in range(TT_PER_GROUP):
                nc.sync.dma_start(out=x_raw[:, 0, tt, :], in_=xv[0, :, tt, :])
            for gi in range(1, NG):
                nc.sync.dma_start(out=x_raw[:, gi, :, :], in_=xv[gi])
            nc.scalar.dma_start(out=u_sb[:], in_=u_in[:])
            nc.scalar.dma_start(out=wt_sb[:], in_=wt_in[:])
            make_identity(nc, ident)
            make_identity(nc, ident16)
            nc.vector.memset(ones[:], 1.0)
            # gate bias rides in g (col 72 of kb 0); cast once to fp32
            gb_sb = singles.tile([E, 1], f32)
            nc.vector.tensor_copy(gb_sb[:], g_sb[0:E, 0, GCOLS:GW])

            def phase_a_tr(gi, kb_lo, kb_hi):
                """PE-transpose x k-blocks into xT (fp16 ident: 1 cyc/row)."""
                for kb in range(kb_lo, kb_hi):
                    pt = ptp.tile([128, TG], f16, tag="pt")
                    for tt in range(TT_PER_GROUP):
                        nc.tensor.transpose(
                            pt[:, ts(tt, 128)],
                            x_raw[:, gi, tt, ds(kb * 128, 128)],
                            ident16,
                        )
                    nc.scalar.copy(xT[:, gi, kb, :], pt[:])

            def phase_a_mm(gi, pg, kb_lo, kb_hi):
                """Merged down+gate matmul for k-blocks [kb_lo, kb_hi)."""
                for kb in range(kb_lo, kb_hi):
                    nc.tensor.matmul(
                        pg[:], g_sb[:, kb, 0:GCOLS], xT[:, gi, kb, :],
                        start=(kb == 0), stop=(kb == 7),
                    )

            def phase_a_extract(pg):
                """Pull down-projection + biased logits out of PSUM."""
                hr = smallp.tile([ER, TG], f32, tag="hr")
                # non-zero partition bases are capped at 32 partitions: split
                nc.scalar.copy(hr[0:32, :], pg[DN0:DN0 + 32, :])
                nc.scalar.copy(hr[32:ER, :], pg[DN0 + 32:DN0 + ER, :])
                lt3 = smallp.tile([E, TG], f32, tag="lt3")
                nc.vector.tensor_scalar_add(lt3[:], pg[0:E, :], gb_sb[:])
                return hr, lt3

            def phase_a0_tr(gi, tt):
                """tt-major transposes (prologue): one 256KB x chunk feeds
                a full token-tile column of xT."""
                for half in range(2):
                    pt = ptp.tile([128, TG], f16, tag="pt")
                    for k in range(4):
                        kb = half * 4 + k
                        nc.tensor.transpose(
                            pt[:, ts(k, 128)],
                            x_raw[:, gi, tt, ds(kb * 128, 128)],
                            ident16,
                        )
                    nc.scalar.copy(
                        xT[:, gi, ds(half * 4, 4), ds(tt * 128, 128)],
                        pt[:],
                    )

            def phase_sl(lt3):
                """Transpose logits to [token, e] (PE, tiny)."""
                psl = pslp.tile([128, TT_PER_GROUP, E], f32, tag="psl")
                for tt in range(TT_PER_GROUP):
                    nc.tensor.transpose(
                        psl[:, tt, :], lt3[:, ts(tt, 128)], ident[0:E, 0:E],
                    )
                return psl

            def phase_sm(psl):
                """Top-2 softmax over e, tokens on partitions; returns w40."""
                L = smallp.tile([128, TT_PER_GROUP, E], f32, tag="L")
                nc.vector.tensor_copy(L[:], psl[:])
                m1 = smallp.tile([128, TT_PER_GROUP], f32, tag="m1")
                nc.vector.reduce_max(m1[:], L[:], axis=mybir.AxisListType.X)
                Lm = smallp.tile([128, TT_PER_GROUP, E], f32, tag="Lm")
                nc.vector.tensor_tensor(
                    Lm[:], L[:], m1[:, :, None].to_broadcast(L.shape),
                    mybir.AluOpType.subtract,
                )
                mmax = smallp.tile([128, TT_PER_GROUP, E], f32, tag="mmax")
                nc.vector.tensor_scalar(
                    mmax[:], Lm[:], 0.0, None, op0=mybir.AluOpType.is_equal,
                )
                nc.vector.tensor_scalar_mul(mmax[:], mmax[:], -1e30)
                nc.vector.tensor_tensor(mmax[:], Lm[:], mmax[:], mybir.AluOpType.add)
                m2 = smallp.tile([128, TT_PER_GROUP], f32, tag="m2")
                nc.vector.reduce_max(m2[:], mmax[:], axis=mybir.AxisListType.X)
                mask2 = smallp.tile([128, TT_PER_GROUP, E], f32, tag="mask2")
                nc.vector.tensor_tensor(
                    mask2[:], Lm[:], m2[:, :, None].to_broadcast(Lm.shape),
                    mybir.AluOpType.is_ge,
                )
                ex = smallp.tile([128, TT_PER_GROUP, E], f32, tag="ex")
                nc.scalar.activation(ex[:], Lm[:], mybir.ActivationFunctionType.Exp)
                nc.vector.tensor_tensor(ex[:], ex[:], mask2[:], mybir.AluOpType.mult)
                zsum = smallp.tile([128, TT_PER_GROUP], f32, tag="zsum")
                nc.vector.reduce_sum(zsum[:], ex[:], axis=mybir.AxisListType.X)
                nc.vector.reciprocal(zsum[:], zsum[:])
                wfull = smallp.tile([128, TT_PER_GROUP, E], f32, tag="wfull")
                nc.vector.tensor_tensor(
                    wfull[:], ex[:], zsum[:, :, None].to_broadcast(ex.shape),
                    mybir.AluOpType.mult,
                )
                w40 = smallp.tile([128, TT_PER_GROUP, ER], f32, tag="w40")
                nc.vector.tensor_copy(
                    w40[:],
                    wfull[:, :, :, None].to_broadcast([128, TT_PER_GROUP, E, R]),
                )
                return w40

            def phase_psw(w40):
                """Transpose gate weights to [er, token] (PE)."""
                psw = pswp.tile([ER, TG], f32, tag="psw")
                for tt in range(TT_PER_GROUP):
                    nc.tensor.transpose(psw[:, ts(tt, 128)], w40[:, tt, :], ident)
                return psw

            def phase_c_base(gi, tt):
                """The 16 base matmuls for one token tile (psum left open)."""
                o_sb = outp.tile([128, D], f32, tag="o_sb")
                pout0 = pop.tile([128, 512], f32, tag="po")
                pout1 = pop.tile([128, 512], f32, tag="po")
                pouts = [pout0, pout1]
                for kb in range(8):
                    for ch in range(2):
                        nc.tensor.matmul(
                            pouts[ch][:], xT[:, gi, kb, ts(tt, 128)],
                            wt_sb[:, kb, ds(ch * 512, 512)],
                            start=(kb == 0), stop=False,
                        )
                return o_sb, pouts

            def phase_c_finish(gi, tt, hr, psw, o_sb, pouts):
                """LoRA weighting, u matmuls, copy-out, store."""
                h1 = h1p.tile([ER + 1, 128], f16, tag="h1")
                nc.vector.tensor_copy(h1[:], ones.to_broadcast([ER + 1, 128]))
                nc.vector.tensor_tensor(
                    h1[0:ER, :], hr[:, ts(tt, 128)], psw[:, ts(tt, 128)],
                    mybir.AluOpType.mult,
                )
                for ch in range(2):
                    nc.tensor.matmul(
                        pouts[ch][:], h1[:], u_sb[:, ds(ch * 512, 512)],
                        start=False, stop=True,
                    )
                    # ch0 copy on Scalar, ch1 on Vector — balances engine
                    # load; stores split across both DMA queues so the two
                    # transfers drain in parallel (Sync is idle by then)
                    if ch == 0:
                        nc.scalar.copy(o_sb[:, ds(0, 512)], pouts[0][:])
                        dma_eng = nc.sync
                    else:
                        nc.vector.tensor_copy(o_sb[:, ds(512, 512)], pouts[1][:])
                        dma_eng = nc.scalar
                    dma_eng.dma_start(
                        out=out_dram[ds(gi * TG + tt * 128, 128), ds(ch * 512, 512)],
                        in_=o_sb[:, ds(ch * 512, 512)],
                    )

            def phase_c_tt(gi, tt, hr, psw):
                o_sb, pouts = phase_c_base(gi, tt)
                phase_c_finish(gi, tt, hr, psw, o_sb, pouts)

            # ---- software-pipelined emission ----
            # Prologue: A(0)'s transposes are paced by the four x(g0) chunk
            # arrivals; C(0).tt0's base matmuls (which need only wt + tt0's
            # xT columns) are hoisted into those wait windows. A(g+1) is
            # split across C(g)'s token tiles; psw(g) waits on a softmax
            # emitted a full group earlier, so the Vector chain never gates
            # the PE.
            hr_lt = {}
            w40s = {}
            phase_a0_tr(0, 0)
            phase_a0_tr(0, 1)
            c00 = phase_c_base(0, 0)
            phase_a0_tr(0, 2)
            phase_a0_tr(0, 3)
            pg0 = pgp.tile([GCOLS, TG], f32, tag="pg")
            phase_a_mm(0, pg0, 0, 8)
            hr_lt[0] = phase_a_extract(pg0)
            w40s[0] = phase_sm(phase_sl(hr_lt[0][1]))
            for gi in range(NG):
                hr, _ = hr_lt[gi]
                psw = phase_psw(w40s[gi])
                pg_next = None
                if gi + 1 < NG:
                    pg_next = pgp.tile([GCOLS, TG], f32, tag="pg")
                if gi == 0:
                    phase_c_finish(0, 0, hr, psw, *c00)
                else:
                    phase_c_tt(gi, 0, hr, psw)
                if pg_next is not None:
                    phase_a_tr(gi + 1, 0, 4)
                    phase_a_mm(gi + 1, pg_next, 0, 4)
                phase_c_tt(gi, 1, hr, psw)
                if pg_next is not None:
                    phase_a_tr(gi + 1, 4, 8)
                    phase_a_mm(gi + 1, pg_next, 4, 8)
                    hr_lt[gi + 1] = phase_a_extract(pg_next)
                    w40s[gi + 1] = phase_sm(phase_sl(hr_lt[gi + 1][1]))
                phase_c_tt(gi, 2, hr, psw)
                phase_c_tt(gi, 3, hr, psw)

    nc.compile()
    return nc


def pack_weights(W_base, b_base, gate_W, gate_b, lora_down, lora_up):
    """Host-side packing of the replicated weights into device layouts."""
    W_base = np.asarray(W_base, np.float32)
    b_base = np.asarray(b_base, np.float32)
    gate_W = np.asarray(gate_W, np.float32)
    gate_b = np.asarray(gate_b, np.float32)
    lora_down = np.asarray(lora_down, np.float32)
    lora_up = np.asarray(lora_up, np.float32)

    # wt[p, kb, o] = W^T[d, o] = W_base[o, d], d = kb*128+p
    wt = np.ascontiguousarray(
        np.ascontiguousarray(W_base.T).reshape(8, 128, D).transpose(1, 0, 2)
    ).astype(np.float16)
    # merged lhsT: cols 0..9 gate_W^T, cols 32..71 lora_down^T (32-aligned);
    # col 72 of kb 0 rows 0..9 carries gate_b
    G = np.zeros((D, GW), np.float32)
    G[:, 0:E] = gate_W.T
    G[:, DN0:DN0 + ER] = lora_down.reshape(ER, D).T
    g = np.ascontiguousarray(G.reshape(8, 128, GW).transpose(1, 0, 2)).astype(np.float16)
    g[0:E, 0, GCOLS] = gate_b.astype(np.float16)
    # u rows 0..39: lora_up[e, o, r]*0.25 -> [er, o]; row 40: b_base
    U = lora_up.transpose(0, 2, 1).reshape(ER, D) * (1.0 / R)
    u = np.ascontiguousarray(np.concatenate([U, b_base[None, :]], axis=0)).astype(np.float16)
    return {"wt": wt, "g": g, "u": u}


def run(nc, inputs, Tc, n_cores=8, trace=False):
    """Shard x over cores, run SPMD, gather output."""
    from concourse.bass_utils import run_bass_kernel_spmd

    x = np.asarray(inputs["x"], np.float32)
    B, S, _ = x.shape
    xf = x.reshape(B * S, D).astype(np.float16)
    assert B * S == Tc * n_cores
    packed = pack_weights(
        inputs["W_base"], inputs["b_base"], inputs["gate_W"],
        inputs["gate_b"], inputs["lora_down"], inputs["lora_up"],
    )
    in_maps = [
        {"x": np.ascontiguousarray(xf[c * Tc:(c + 1) * Tc]), **packed}
        for c in range(n_cores)
    ]
    kwargs = {}
    if trace:
        install_ntff_hook()
        kwargs = {"trace": True}
    res = run_bass_kernel_spmd(nc, in_maps, core_ids=list(range(n_cores)), **kwargs)
    out = np.concatenate([res.results[c]["out"] for c in range(n_cores)], axis=0)
    return out.reshape(B, S, D), res


_NC_CACHE = {}


def kernel(**inputs):
    """Full-input MoE-LoRA forward on 8 TRN2 NeuronCores (token-parallel).

    Takes the unsharded inputs from setup_inputs(), returns [B, S, D] fp32.
    """
    x = np.asarray(inputs["x"], np.float32)
    B, S, _ = x.shape
    n_cores = 8
    total = B * S
    assert total % n_cores == 0
    Tc = total // n_cores
    key = (Tc, n_cores)
    if key not in _NC_CACHE:
        _NC_CACHE[key] = build_kernel(Tc, n_cores=n_cores)
    nc = _NC_CACHE[key]
    last_err = None
    for _ in range(3):  # transient device wedges recover on retry
        try:
            out, _res = run(nc, inputs, Tc, n_cores=n_cores)
            return out
        except Exception as e:  # noqa: BLE001
            last_err = e
            import time as _time
            _time.sleep(5)
    raise last_err


# revision 51
# speedup vs baseline: 1.0311x; 1.0311x over previous
"""MoE-LoRA linear kernel for TRN2, data-parallel over tokens across 8 cores.

Per-core computation (Tc tokens, D=1024, E=10, R=4, TOP_K=2):
  base = x @ W^T + b ; logits = x @ gateW^T + gb ; top2 softmax -> dense w[t,e]
  h = (x @ lora_down^T) * w  (rank-expanded) ; out = base + 0.25 * h @ lora_up^T

Datapath is fp16 (x, W, gate, lora) with fp32 PSUM accumulate — end-to-end
rel err ~1.3e-3 vs the fp32 reference (validated on host). fp16 enables the
DMA xbar transpose, so x streams DRAM->SBUF already transposed per k-block:
no PE transposes, no psum->sbuf copies for xT, and half the x/W HBM traffic.
The down-projection and gate logits share one 50-column stationary matrix
(one matmul per k-block). Softmax stays fp32.

Emission is software-pipelined so the in-order Tensor queue never waits on
the Vector softmax: phase A of group g+2 and the gating transposes of group
g+1 are emitted inside phase C of group g. Inputs ride the Sync DMA queue;
weights + outputs ride the Scalar (Activation) queue.
"""

import contextlib
import ctypes
import sys
import types

import numpy as np

SO_PATH = "/opt/axon/libaxon_pjrt.so"

D = 1024
E = 10
R = 4
ER = E * R          # 40
DN0 = 32            # down block start (32-aligned partition base)
GCOLS = DN0 + ER    # 72 = gate(10) + pad + down(40)@32
GW = GCOLS + 1      # 73: col 72 of kb 0 carries gate_b (avoids a 10x4B
                    # scattered DMA that takes ~10us to complete)
TT_PER_GROUP = 4    # 128-token tiles per 512-token group
TG = 128 * TT_PER_GROUP  # 512 tokens per group


def install_ntff_hook():
    """run_bass_kernel_spmd(trace=True) needs antenv.axon_hooks; synthesize it."""
    if "antenv.axon_hooks" in sys.modules:
        return
    def _ntff_profile_via_ctypes(so_path):
        lib = ctypes.CDLL(so_path)
        if not hasattr(lib, "axon_start_nrt_profile"):
            return None
        lib.axon_start_nrt_profile.argtypes = [ctypes.POINTER(ctypes.c_int64), ctypes.c_size_t]
        lib.axon_start_nrt_profile.restype = ctypes.c_int64
        lib.axon_stop_nrt_profile.argtypes = [ctypes.c_char_p]
        lib.axon_stop_nrt_profile.restype = ctypes.c_int64

        @contextlib.contextmanager
        def _hook(output_dir, device_ids):
            import jax
            jax.devices()
            if device_ids:
                ids = (ctypes.c_int64 * len(device_ids))(*device_ids)
                rc = lib.axon_start_nrt_profile(ids, len(device_ids))
            else:
                rc = lib.axon_start_nrt_profile(None, 0)
            if rc != 0:
                raise RuntimeError(f"axon_start_nrt_profile rc={rc}")
            try:
                yield
            finally:
                n = lib.axon_stop_nrt_profile(str(output_dir).encode())
                if n < 0:
                    raise RuntimeError(f"axon_stop_nrt_profile rc={n}")
        return _hook

    mod = types.ModuleType("antenv.axon_hooks")
    mod.get_axon_ntff_profile_hook = lambda: _ntff_profile_via_ctypes(SO_PATH)
    sys.modules["antenv.axon_hooks"] = mod


def build_kernel(Tc, n_cores=8):
    import concourse.bass as bass  # noqa: F401
    import concourse.mybir as mybir
    import concourse.tile as tile
    from concourse import bacc
    from concourse.bass import ds, ts
    from concourse.masks import make_identity

    f16 = mybir.dt.float16
    f32 = mybir.dt.float32
    NG = Tc // TG  # groups of 512 tokens
    assert Tc % TG == 0

    nc = bacc.Bacc("TRN2", target_bir_lowering=False, debug=False, num_devices=n_cores)

    x_in = nc.declare_dram_parameter("x", [Tc, D], f16, isOutput=False)
    wt_in = nc.declare_dram_parameter("wt", [128, 8, D], f16, isOutput=False)
    g_in = nc.declare_dram_parameter("g", [128, 8, GW], f16, isOutput=False)
    u_in = nc.declare_dram_parameter("u", [ER + 1, D], f16, isOutput=False)
    out_dram = nc.declare_dram_parameter("out", [Tc, D], f32, isOutput=True)

    with tile.TileContext(nc) as tc:
        with contextlib.ExitStack() as ctx:
            singles = ctx.enter_context(tc.tile_pool(name="singles", bufs=1))
            smallp = ctx.enter_context(tc.tile_pool(name="smallp", bufs=2))
            h1p = ctx.enter_context(tc.tile_pool(name="h1p", bufs=3))
            outp = ctx.enter_context(tc.tile_pool(name="outp", bufs=4))
            pgp = ctx.enter_context(tc.tile_pool(name="pgp", bufs=1, space="PSUM"))
            pslp = ctx.enter_context(tc.tile_pool(name="pslp", bufs=1, space="PSUM"))
            pswp = ctx.enter_context(tc.tile_pool(name="pswp", bufs=1, space="PSUM"))
            pop = ctx.enter_context(tc.tile_pool(name="pop", bufs=3, space="PSUM"))
            ptp = ctx.enter_context(tc.tile_pool(name="ptp", bufs=2, space="PSUM"))

            # ---- constants / persistent tensors ----
            wt_sb = singles.tile([128, 8, D], f16)
            g_sb = singles.tile([128, 8, GW], f16)
            u_sb = singles.tile([ER + 1, D], f16)
            ident = singles.tile([128, 128], f32)
            ident16 = singles.tile([128, 128], f16)
            ones = singles.tile([ER + 1, 1], f32)
            xT = singles.tile([128, NG, 8, TG], f16)
            x_raw = singles.tile([128, NG, TT_PER_GROUP, D], f16)

            # inputs on the Sync DMA queue; weights+outputs on Scalar's queue
            # Plain contiguous loads only (DMA xbar transposes serialize on
            # completion and collapse under bandwidth contention): x tokens on
            # partitions via Sync queue; weights + outputs on Scalar queue.
            nc.sync.dma_start(out=g_sb[:], in_=g_in[:])
            xv = x_in.rearrange("(ng a p) d -> ng p a d", p=128, a=TT_PER_GROUP)
            # group 0 lands per-tt so the first transposes start ~10us sooner
            for tt in range(TT_PER_GROUP):
                nc.sync.dma_start(out=x_raw[:, 0, tt, :], in_=xv[0, :, tt, :])
            for gi in range(1, NG):
                nc.sync.dma_start(out=x_raw[:, gi, :, :], in_=xv[gi])
            nc.scalar.dma_start(out=u_sb[:], in_=u_in[:])
            nc.scalar.dma_start(out=wt_sb[:], in_=wt_in[:])
            make_identity(nc, ident)
            make_identity(nc, ident16)
            nc.vector.memset(ones[:], 1.0)
            # gate bias rides in g (col 72 of kb 0); cast once to fp32
            gb_sb = singles.tile([E, 1], f32)
            nc.vector.tensor_copy(gb_sb[:], g_sb[0:E, 0, GCOLS:GW])

            def phase_a_tr(gi, kb_lo, kb_hi):
                """PE-transpose x k-blocks into xT (fp16 ident: 1 cyc/row)."""
                for kb in range(kb_lo, kb_hi):
                    pt = ptp.tile([128, TG], f16, tag="pt")
                    for tt in range(TT_PER_GROUP):
                        nc.tensor.transpose(
                            pt[:, ts(tt, 128)],
                            x_raw[:, gi, tt, ds(kb * 128, 128)],
                            ident16,
                        )
                    nc.scalar.copy(xT[:, gi, kb, :], pt[:])

            def phase_a_mm(gi, pg, kb_lo, kb_hi):
                """Merged down+gate matmul for k-blocks [kb_lo, kb_hi)."""
                for kb in range(kb_lo, kb_hi):
                    nc.tensor.matmul(
                        pg[:], g_sb[:, kb, 0:GCOLS], xT[:, gi, kb, :],
                        start=(kb == 0), stop=(kb == 7),
                    )

            def phase_a_extract(pg):
                """Pull down-projection + biased logits out of PSUM."""
                hr = smallp.tile([ER, TG], f32, tag="hr")
                # non-zero partition bases are capped at 32 partitions: split
                nc.scalar.copy(hr[0:32, :], pg[DN0:DN0 + 32, :])
                nc.scalar.copy(hr[32:ER, :], pg[DN0 + 32:DN0 + ER, :])
                lt3 = smallp.tile([E, TG], f32, tag="lt3")
                nc.vector.tensor_scalar_add(lt3[:], pg[0:E, :], gb_sb[:])
                return hr, lt3

            def phase_a(gi):
                """Prologue-only variant: tt-major transposes, so the first
                PE work needs just one 256KB x chunk, not the whole group."""
                pg = pgp.tile([GCOLS, TG], f32, tag="pg")
                for tt in range(TT_PER_GROUP):
                    for half in range(2):
                        pt = ptp.tile([128, TG], f16, tag="pt")
                        for k in range(4):
                            kb = half * 4 + k
                            nc.tensor.transpose(
                                pt[:, ts(k, 128)],
                                x_raw[:, gi, tt, ds(kb * 128, 128)],
                                ident16,
                            )
                        nc.scalar.copy(
                            xT[:, gi, ds(half * 4, 4), ds(tt * 128, 128)],
                            pt[:],
                        )
                phase_a_mm(gi, pg, 0, 8)
                return phase_a_extract(pg)

            def phase_sl(lt3):
                """Transpose logits to [token, e] (PE, tiny)."""
                psl = pslp.tile([128, TT_PER_GROUP, E], f32, tag="psl")
                for tt in range(TT_PER_GROUP):
                    nc.tensor.transpose(
                        psl[:, tt, :], lt3[:, ts(tt, 128)], ident[0:E, 0:E],
                    )
                return psl

            def phase_sm(psl):
                """Top-2 softmax over e, tokens on partitions; returns w40."""
                L = smallp.tile([128, TT_PER_GROUP, E], f32, tag="L")
                nc.vector.tensor_copy(L[:], psl[:])
                m1 = smallp.tile([128, TT_PER_GROUP], f32, tag="m1")
                nc.vector.reduce_max(m1[:], L[:], axis=mybir.AxisListType.X)
                Lm = smallp.tile([128, TT_PER_GROUP, E], f32, tag="Lm")
                nc.vector.tensor_tensor(
                    Lm[:], L[:], m1[:, :, None].to_broadcast(L.shape),
                    mybir.AluOpType.subtract,
                )
                mmax = smallp.tile([128, TT_PER_GROUP, E], f32, tag="mmax")
                nc.vector.tensor_scalar(
                    mmax[:], Lm[:], 0.0, None, op0=mybir.AluOpType.is_equal,
                )
                nc.vector.tensor_scalar_mul(mmax[:], mmax[:], -1e30)
                nc.vector.tensor_tensor(mmax[:], Lm[:], mmax[:], mybir.AluOpType.add)
                m2 = smallp.tile([128, TT_PER_GROUP], f32, tag="m2")
                nc.vector.reduce_max(m2[:], mmax[:], axis=mybir.AxisListType.X)
                mask2 = smallp.tile([128, TT_PER_GROUP, E], f32, tag="mask2")
                nc.vector.tensor_tensor(
                    mask2[:], Lm[:], m2[:, :, None].to_broadcast(Lm.shape),
                    mybir.AluOpType.is_ge,
                )
                ex = smallp.tile([128, TT_PER_GROUP, E], f32, tag="ex")
                nc.scalar.activation(ex[:], Lm[:], mybir.ActivationFunctionType.Exp)
                nc.vector.tensor_tensor(ex[:], ex[:], mask2[:], mybir.AluOpType.mult)
                zsum = smallp.tile([128, TT_PER_GROUP], f32, tag="zsum")
                nc.vector.reduce_sum(zsum[:], ex[:], axis=mybir.AxisListType.X)
                nc.vector.reciprocal(zsum[:], zsum[:])
                wfull = smallp.tile([128, TT_PER_GROUP, E], f32, tag="wfull")
                nc.vector.tensor_tensor(
                    wfull[:], ex[:], zsum[:, :, None].to_broadcast(ex.shape),
                    mybir.AluOpType.mult,
                )
                w40 = smallp.tile([128, TT_PER_GROUP, ER], f32, tag="w40")
                nc.vector.tensor_copy(
                    w40[:],
                    wfull[:, :, :, None].to_broadcast([128, TT_PER_GROUP, E, R]),
                )
                return w40

            def phase_psw(w40):
                """Transpose gate weights to [er, token] (PE)."""
                psw = pswp.tile([ER, TG], f32, tag="psw")
                for tt in range(TT_PER_GROUP):
                    nc.tensor.transpose(psw[:, ts(tt, 128)], w40[:, tt, :], ident)
                return psw

            def phase_c_tt(gi, tt, hr, psw, mid_hook=None):
                """One 128-token tile: base+LoRA matmuls, copy out, store.

                mid_hook (if set) is emitted between the base matmuls and the
                h1/u matmuls — used to slot the NEXT group's psw transpose
                into the PE queue as late as its consumer allows, maximizing
                the Vector softmax's head start.
                """
                o_sb = outp.tile([128, D], f32, tag="o_sb")
                pout0 = pop.tile([128, 512], f32, tag="po")
                pout1 = pop.tile([128, 512], f32, tag="po")
                pouts = [pout0, pout1]
                for kb in range(8):
                    for ch in range(2):
                        nc.tensor.matmul(
                            pouts[ch][:], xT[:, gi, kb, ts(tt, 128)],
                            wt_sb[:, kb, ds(ch * 512, 512)],
                            start=(kb == 0), stop=False,
                        )
                ret = mid_hook() if mid_hook is not None else None
                h1 = h1p.tile([ER + 1, 128], f16, tag="h1")
                nc.vector.tensor_copy(h1[:], ones.to_broadcast([ER + 1, 128]))
                nc.vector.tensor_tensor(
                    h1[0:ER, :], hr[:, ts(tt, 128)], psw[:, ts(tt, 128)],
                    mybir.AluOpType.mult,
                )
                for ch in range(2):
                    nc.tensor.matmul(
                        pouts[ch][:], h1[:], u_sb[:, ds(ch * 512, 512)],
                        start=False, stop=True,
                    )
                    # ch0 copy on Scalar, ch1 on Vector — balances engine
                    # load; stores split across both DMA queues so the two
                    # transfers drain in parallel (Sync is idle by then)
                    if ch == 0:
                        nc.scalar.copy(o_sb[:, ds(0, 512)], pouts[0][:])
                        dma_eng = nc.sync
                    else:
                        nc.vector.tensor_copy(o_sb[:, ds(512, 512)], pouts[1][:])
                        dma_eng = nc.scalar
                    dma_eng.dma_start(
                        out=out_dram[ds(gi * TG + tt * 128, 128), ds(ch * 512, 512)],
                        in_=o_sb[:, ds(ch * 512, 512)],
                    )
                return ret

            # ---- software-pipelined emission ----
            # A(g+1) (transposes + merged matmuls) is split across C(g)'s
            # token tiles; psw(g) waits on a softmax emitted a full group
            # earlier, so the Vector chain never gates the PE.
            hr_lt = {}
            w40s = {}
            hr_lt[0] = phase_a(0)
            w40s[0] = phase_sm(phase_sl(hr_lt[0][1]))
            for gi in range(NG):
                hr, _ = hr_lt[gi]
                psw = phase_psw(w40s[gi])
                pg_next = None
                if gi + 1 < NG:
                    pg_next = pgp.tile([GCOLS, TG], f32, tag="pg")
                phase_c_tt(gi, 0, hr, psw)
                if pg_next is not None:
                    phase_a_tr(gi + 1, 0, 4)
                    phase_a_mm(gi + 1, pg_next, 0, 4)
                phase_c_tt(gi, 1, hr, psw)
                if pg_next is not None:
                    phase_a_tr(gi + 1, 4, 8)
                    phase_a_mm(gi + 1, pg_next, 4, 8)
                    hr_lt[gi + 1] = phase_a_extract(pg_next)
                    w40s[gi + 1] = phase_sm(phase_sl(hr_lt[gi + 1][1]))
                phase_c_tt(gi, 2, hr, psw)
                phase_c_tt(gi, 3, hr, psw)

    nc.compile()
    return nc


def pack_weights(W_base, b_base, gate_W, gate_b, lora_down, lora_up):
    """Host-side packing of the replicated weights into device layouts."""
    W_base = np.asarray(W_base, np.float32)
    b_base = np.asarray(b_base, np.float32)
    gate_W = np.asarray(gate_W, np.float32)
    gate_b = np.asarray(gate_b, np.float32)
    lora_down = np.asarray(lora_down, np.float32)
    lora_up = np.asarray(lora_up, np.float32)

    # wt[p, kb, o] = W^T[d, o] = W_base[o, d], d = kb*128+p
    wt = np.ascontiguousarray(
        np.ascontiguousarray(W_base.T).reshape(8, 128, D).transpose(1, 0, 2)
    ).astype(np.float16)
    # merged lhsT: cols 0..9 gate_W^T, cols 32..71 lora_down^T (32-aligned);
    # col 72 of kb 0 rows 0..9 carries gate_b
    G = np.zeros((D, GW), np.float32)
    G[:, 0:E] = gate_W.T
    G[:, DN0:DN0 + ER] = lora_down.reshape(ER, D).T
    g = np.ascontiguousarray(G.reshape(8, 128, GW).transpose(1, 0, 2)).astype(np.float16)
    g[0:E, 0, GCOLS] = gate_b.astype(np.float16)
    # u rows 0..39: lora_up[e, o, r]*0.25 -> [er, o]; row 40: b_base
    U = lora_up.transpose(0, 2, 1).reshape(ER, D) * (1.0 / R)
    u = np.ascontiguousarray(np.concatenate([U, b_base[None, :]], axis=0)).astype(np.float16)
    return {"wt": wt, "g": g, "u": u}


def run(nc, inputs, Tc, n_cores=8, trace=False):
    """Shard x over cores, run SPMD, gather output."""
    from concourse.bass_utils import run_bass_kernel_spmd

    x = np.asarray(inputs["x"], np.float32)
    B, S, _ = x.shape
    xf = x.reshape(B * S, D).astype(np.float16)
    assert B * S == Tc * n_cores
    packed = pack_weights(
        inputs["W_base"], inputs["b_base"], inputs["gate_W"],
        inputs["gate_b"], inputs["lora_down"], inputs["lora_up"],
    )
    in_maps = [
        {"x": np.ascontiguousarray(xf[c * Tc:(c + 1) * Tc]), **packed}
        for c in range(n_cores)
    ]
    kwargs = {}
    if trace:
        install_ntff_hook()
        kwargs = {"trace": True}
    res = run_bass_kernel_spmd(nc, in_maps, core_ids=list(range(n_cores)), **kwargs)
    out = np.concatenate([res.results[c]["out"] for c in range(n_cores)], axis=0)
    return out.reshape(B, S, D), res


_NC_CACHE = {}


def kernel(**inputs):
    """Full-input MoE-LoRA forward on 8 TRN2 NeuronCores (token-parallel).

    Takes the unsharded inputs from setup_inputs(), returns [B, S, D] fp32.
    """
    x = np.asarray(inputs["x"], np.float32)
    B, S, _ = x.shape
    n_cores = 8
    total = B * S
    assert total % n_cores == 0
    Tc = total // n_cores
    key = (Tc, n_cores)
    if key not in _NC_CACHE:
        _NC_CACHE[key] = build_kernel(Tc, n_cores=n_cores)
    nc = _NC_CACHE[key]
    last_err = None
    for _ in range(3):  # transient device wedges recover on retry
        try:
            out, _res = run(nc, inputs, Tc, n_cores=n_cores)
            return out
        except Exception as e:  # noqa: BLE001
            last_err = e
            import time as _time
            _time.sleep(5)
    raise last_err
